# revision 59
# baseline (speedup 1.0000x reference)
"""Content-based addressing read (DNC-style) for Trainium2.

Computes softmax_n( strengths[r] * cos_sim(memory[b,n,:], read_vectors[b,:,r]) )
for B=16, N=32768, W=128, R=8, sharded batch-parallel across 8 NeuronCores
(2 batches per core).

v18: norm reduction moved onto the PE.
  - The w-reduction for ||mem_n||^2 no longer runs as squares+fold-tree on
    the vector engines. Instead the *drained* memT chunks are squared
    (GpSimd/ACT/DVE rotation, SBUF->SBUF) and a per-tile ones-column matmul
    accumulates sum_w memT^2 directly into PSUM in [p, t] layout (f32
    accumulate — better precision than the old f16 fold tree). This removes
    the entire n-major square pass + fd1/fd2/fd3/reduce chain from DVE/ACT.
  - norm matmuls are deferred one group so the PE queue never waits on the
    square producers; they also keep the PE busier (p-state ramp).
  - inv_nrm = ACT Abs_reciprocal_sqrt straight from the norm PSUM tile.
  - scores stored [128, R, T]: the sim-PSUM drain is a fused DVE multiply
    by inv_nrm; s1 reduce is contiguous; output written f16 scaled by 2^15
    (host rescales), quarters issued on sync/scalar/gpsimd HWDGE queues.
  - gpsimd casting DMA (f32 HBM -> f16 SBUF) in 2 half-group pieces with a
    5-group issue lookahead.
Softmax math stays fp32; no max subtraction (|scores| <= ~1.2); the
reference's +1e-8 is a provable fp32 no-op (normalizer ~128).

Output is stored in DRAM as (b, p, r, tau) f16*2^15 with n = g*4096 + p*32 + t,
tau = g*32 + t; the host rescales and re-transposes to (b, n, r) f32.
"""

import sys

for _p in ("/opt/trn_rl_repo",):
    if _p not in sys.path:
        sys.path.insert(0, _p)

from contextlib import ExitStack

import numpy as np

import concourse.bass as bass
import concourse.bacc as bacc
import concourse.tile as tile
from concourse import mybir
from concourse import bass_isa
from concourse.bass_utils import run_bass_kernel_spmd

F32 = mybir.dt.float32
F16 = mybir.dt.float16
AF = mybir.ActivationFunctionType

B, N, W, R = 16, 32768, 128, 8
NCORES = 8
BLOC = B // NCORES          # batches per core
T = N // 128                # 256 n-tiles of 128 per batch
NG = 8                      # DMA groups per batch
TPG = T // NG               # 32 tiles per group (4096 n, 2MB)
CH = 8                      # tiles per PSUM transpose chunk (1024 cols)
NCH = TPG // CH             # chunks per group
NSTEP = BLOC * NG           # 16 flat steps

# ---- tuning knobs ----
MEMT_DRAIN = "vs"           # rotation for memT PSUM->SBUF drains (no gp: PSUM)
SQ_ROT = "gad"              # rotation for memT^2 squares (g=GpSimd a=ACT d=DVE)
DMA_SPLIT = 2               # casting DMAs per group (earlier consumer wakeup)
DMA_AHEAD = 5               # DMA issue lookahead (must be < IN_BUFS - 2)
IN_BUFS = 10
OUT_SCALE = 32768.0         # output written f16 * 2^15; host divides


def build_program():
    nc = bacc.Bacc("TRN2", target_bir_lowering=False, debug=False, num_devices=NCORES)

    mem = nc.dram_tensor("memory", [BLOC, N, W], F32, kind="ExternalInput").ap()
    rv = nc.dram_tensor("read_vectors", [BLOC, W, R], F32, kind="ExternalInput").ap()
    rs = nc.dram_tensor("read_strengths", [BLOC, R], F32, kind="ExternalInput").ap()
    ident = nc.dram_tensor("identity", [128, 128], F32, kind="ExternalInput").ap()
    ones = nc.dram_tensor("ones", [128, 128], F32, kind="ExternalInput").ap()
    out = nc.dram_tensor("out", [BLOC, 128, R, T], F16, kind="ExternalOutput").ap()

    with ExitStack() as ctx:
        tc = ctx.enter_context(tile.TileContext(nc))

        const_pool = ctx.enter_context(tc.tile_pool(name="const", bufs=1))
        id_t = const_pool.tile([128, 128], F32)
        nc.sync.dma_start(id_t[:], ident)
        ones_t = const_pool.tile([128, 128], F32)
        nc.sync.dma_start(ones_t[:], ones)
        id_h = const_pool.tile([128, 128], F16)
        nc.scalar.copy(id_h[:], id_t[:])
        ones_h = const_pool.tile([128, 128], F16)
        nc.scalar.copy(ones_h[:], ones_t[:])

        in_pool = ctx.enter_context(tc.tile_pool(name="mem_in", bufs=IN_BUFS))
        mtps_pool = ctx.enter_context(tc.tile_pool(name="mtps", bufs=3, space="PSUM"))
        mt_pool = ctx.enter_context(tc.tile_pool(name="mt", bufs=6))
        mtq_pool = ctx.enter_context(tc.tile_pool(name="mtq", bufs=9))
        scps_pool = ctx.enter_context(tc.tile_pool(name="scps", bufs=3, space="PSUM"))
        nrm_pool = ctx.enter_context(tc.tile_pool(name="nrm", bufs=1, space="PSUM"))
        rtps_pool = ctx.enter_context(tc.tile_pool(name="rtps", bufs=1, space="PSUM"))
        smalls = ctx.enter_context(tc.tile_pool(name="smalls", bufs=2))
        score_pool = ctx.enter_context(tc.tile_pool(name="scores", bufs=2))
        scout_pool = ctx.enter_context(tc.tile_pool(name="scout", bufs=2))
        inv_pool = ctx.enter_context(tc.tile_pool(name="inv", bufs=2))

        state = {"drain_i": 0, "sq_i": 0}

        # per-batch / per-step state
        scores_t = [None] * BLOC
        inv_t = [None] * BLOC
        rvp_t = [None] * BLOC
        inv_tot_t = [None] * BLOC
        mem_tiles = {}   # flat step -> mem_g tile
        mtq_tiles = {}   # flat step -> list of squared memT chunks
        scps_tiles = {}  # flat step -> sim psum tile
        nrm_tiles = {}   # flat step -> norm psum tile

        def issue_dma(step):
            b, g = divmod(step, NG)
            mem_g = in_pool.tile([128, TPG, W], F16)
            src = mem[b, g * TPG * 128 : (g + 1) * TPG * 128, :].rearrange(
                "(p t) w -> p t w", p=128
            )
            hp = TPG // DMA_SPLIT
            for k in range(DMA_SPLIT):  # split: consumers wake at sub-group grain
                ts = slice(k * hp, (k + 1) * hp)
                nc.gpsimd.dma_start(mem_g[:, ts, :], src[:, ts, :])
            mem_tiles[step] = mem_g

        def rv_prep(b):
            rv_t = smalls.tile([128, R], F32)
            nc.sync.dma_start(rv_t[:], rv[b])
            rs_t = smalls.tile([1, R], F32)
            nc.sync.dma_start(rs_t[:], rs[b : b + 1, :])
            rs_h = smalls.tile([1, R], F16)
            nc.scalar.copy(rs_h[:], rs_t[:])

            rv2 = smalls.tile([128, R], F16)
            nc.vector.tensor_mul(rv2[:], rv_t[:], rv_t[:])
            nv2_ps = rtps_pool.tile([128, R], F32, tag="prep")
            nc.tensor.matmul(nv2_ps[:], ones_h[:], rv2[:], start=True, stop=True)
            inv_nv = smalls.tile([128, R], F32)
            nc.scalar.activation(inv_nv[:], nv2_ps[:], AF.Abs_reciprocal_sqrt)
            rsb_ps = rtps_pool.tile([128, R], F32, tag="prep")
            nc.tensor.matmul(
                rsb_ps[:], ones_h[0:1, :], rs_h[:], start=True, stop=True
            )
            factor = smalls.tile([128, R], F32)
            nc.vector.tensor_mul(factor[:], rsb_ps[:], inv_nv[:])
            rvp = smalls.tile([128, R], F32, tag="rvp")
            nc.vector.tensor_mul(rvp[:], rv_t[:], factor[:])
            rvp_h = smalls.tile([128, R], F16, tag="rvph")
            nc.scalar.copy(rvp_h[:], rvp[:])
            rvp_t[b] = rvp_h

        def emit_chunk(step, q, scps, sq_list):
            """transposes + memT drain + sim matmuls + memT^2 square."""
            b, g = divmod(step, NG)
            mem_g = mem_tiles[step]
            mt_ps = mtps_pool.tile([128, CH * 128], F16)
            for j in range(CH):
                tt = q * CH + j
                nc.tensor.transpose(
                    mt_ps[:, j * 128 : (j + 1) * 128],
                    mem_g[:, tt, :],
                    id_h[:],
                )
            mt_sb = mt_pool.tile([128, CH * 128], F16)
            de = MEMT_DRAIN[state["drain_i"] % len(MEMT_DRAIN)]
            state["drain_i"] += 1
            if de == "s":
                nc.scalar.copy(mt_sb[:], mt_ps[:])
            else:
                nc.vector.tensor_copy(mt_sb[:], mt_ps[:])

            for j in range(CH):
                tt = q * CH + j
                nc.tensor.matmul(
                    scps[:, tt * R : (tt + 1) * R],
                    mt_sb[:, j * 128 : (j + 1) * 128],
                    rvp_t[b][:],
                    start=True,
                    stop=True,
                )

            # square the drained chunk (SBUF->SBUF, any engine incl. GpSimd)
            mt_sq = mtq_pool.tile([128, CH * 128], F16)
            se = SQ_ROT[state["sq_i"] % len(SQ_ROT)]
            state["sq_i"] += 1
            if se == "g":
                nc.gpsimd.tensor_mul(mt_sq[:], mt_sb[:], mt_sb[:])
            elif se == "a":
                nc.scalar.square(mt_sq[:], mt_sb[:])
            else:
                nc.vector.tensor_mul(mt_sq[:], mt_sb[:], mt_sb[:])
            sq_list.append(mt_sq)

        def emit_norm_mms(step):
            """PE: per-tile ones-column matmuls accumulate ||mem_n||^2 into
            a [128(p), TPG(t)] PSUM tile. Deferred one group so the PE never
            waits on the square producers."""
            sq_list = mtq_tiles.pop(step)
            nrm_ps = nrm_pool.tile([128, TPG], F32)
            for q in range(NCH):
                mt_sq = sq_list[q]
                for j in range(CH):
                    tt = q * CH + j
                    nc.tensor.matmul(
                        nrm_ps[:, tt : tt + 1],
                        mt_sq[:, j * 128 : (j + 1) * 128],
                        ones_h[:, 0:1],
                        start=True,
                        stop=True,
                    )
            nrm_tiles[step] = nrm_ps

        def emit_inv(step):
            """inv_nrm = 1/sqrt(ss) straight from the norm PSUM tile."""
            b, g = divmod(step, NG)
            nrm_ps = nrm_tiles.pop(step)
            nc.scalar.activation(
                inv_t[b][:, g * TPG : (g + 1) * TPG],
                nrm_ps[:],
                AF.Abs_reciprocal_sqrt,
            )

        s1_acc = [None] * BLOC

        def emit_score_drain(step):
            """scores[:, :, tau] = scps * inv_nrm (fused PSUM drain, DVE),
            then exp the slice in place (ACT) and accumulate its s1 partial —
            by the batch tail only the last slice's chain remains."""
            b, g = divmod(step, NG)
            scps = scps_tiles.pop(step)
            scores = scores_t[b]
            ts = slice(g * TPG, (g + 1) * TPG)
            scps_v = scps[:].rearrange("p (t r) -> p t r", r=R).transpose([0, 2, 1])
            inv_b = (
                inv_t[b][:, ts].unsqueeze(1).broadcast_to([128, R, TPG])
            )
            nc.vector.tensor_mul(scores[:, :, ts], scps_v, inv_b)
            nc.scalar.activation(scores[:, :, ts], scores[:, :, ts], AF.Exp)
            part = smalls.tile([128, R], F32, tag="s1part")
            nc.vector.reduce_sum(
                part[:], scores[:, :, ts], axis=mybir.AxisListType.X
            )
            if g == 0:
                acc_new = smalls.tile([128, R], F32, tag="s1acc")
                nc.vector.tensor_copy(acc_new[:], part[:])
                s1_acc[b] = acc_new
            else:
                nc.vector.tensor_add(s1_acc[b][:], s1_acc[b][:], part[:])

        def softmax_tail_a(b):
            """1/total from the accumulated s1 (ACT + PE + DVE)."""
            s1_h = smalls.tile([128, R], F16)
            # pre-scale by 2^-15 so f16 output (x OUT_SCALE) stays normal-range
            nc.scalar.activation(s1_h[:], s1_acc[b][:], AF.Copy, scale=1.0 / OUT_SCALE)
            tot_ps = rtps_pool.tile([128, R], F32, tag="prep")
            nc.tensor.matmul(tot_ps[:], ones_h[:], s1_h[:], start=True, stop=True)
            inv_tot = smalls.tile([128, R], F32, tag="invtot")
            nc.vector.reciprocal_approx_fast(inv_tot[:], tot_ps[:])
            inv_tot_t[b] = inv_tot

        def softmax_tail_b(b):
            """normalize quarters + output DMA on 4 queues (overlap the tail)."""
            scores = scores_t[b]
            inv_tot = inv_tot_t[b]
            scout = scout_pool.tile([128, R, T], F16, tag="scout")
            H = R // 4
            out_eng = [nc.sync, nc.scalar, nc.gpsimd, nc.sync]
            for h in range(4):
                hs = slice(h * H, (h + 1) * H)
                inv_b = inv_tot[:, hs].unsqueeze(2).broadcast_to([128, H, T])
                nc.vector.tensor_mul(scout[:, hs, :], scores[:, hs, :], inv_b)
                out_eng[h].dma_start(out[b, :, hs, :], scout[:, hs, :])

        # prologue: fill DMA pipeline, prep batch 0
        for s0 in range(DMA_AHEAD):
            issue_dma(s0)
        rv_prep(0)

        for step in range(NSTEP):
            b, g = divmod(step, NG)
            if g == 0:
                scores_new = score_pool.tile([128, R, T], F32, tag="scores")
                inv_new = inv_pool.tile([128, T], F32, tag="inv")
                scores_t[b] = scores_new
                inv_t[b] = inv_new
            if step + DMA_AHEAD < NSTEP:
                issue_dma(step + DMA_AHEAD)
            # batch b+1 rv-prep midway through batch b
            if g == NG - 3 and b + 1 < BLOC:
                rv_prep(b + 1)

            # previous group's norm matmuls: ready PE work at the head of the
            # queue (this group's transposes wait on DMA anyway)
            if step >= 1:
                emit_norm_mms(step - 1)
                emit_inv(step - 1)
            # fused score drain: ready DVE work (inv(step-2) emitted at step-1)
            if step >= 2:
                emit_score_drain(step - 2)
            if g == 2 and b > 0:
                softmax_tail_a(b - 1)

            scps = scps_pool.tile([128, TPG * R], F32)
            sq_list = []
            for q in range(NCH):
                emit_chunk(step, q, scps, sq_list)
            scps_tiles[step] = scps
            mtq_tiles[step] = sq_list
            mem_tiles.pop(step)

            if g == 3 and b > 0:
                softmax_tail_b(b - 1)

        # epilogue
        emit_norm_mms(NSTEP - 1)
        emit_inv(NSTEP - 1)
        emit_score_drain(NSTEP - 2)
        emit_score_drain(NSTEP - 1)
        softmax_tail_a(BLOC - 1)
        softmax_tail_b(BLOC - 1)

    nc.compile()
    return nc


_program = None
last_results = None


def _get_program():
    global _program
    if _program is None:
        _program = build_program()
    return _program


def kernel(memory, read_strengths, read_vectors):
    memory = np.asarray(memory, dtype=np.float32)
    read_strengths = np.asarray(read_strengths, dtype=np.float32)
    read_vectors = np.asarray(read_vectors, dtype=np.float32)

    nc = _get_program()
    identity = np.eye(128, dtype=np.float32)
    ones_m = np.ones((128, 128), dtype=np.float32)
    in_maps = []
    for c in range(NCORES):
        sl = slice(c * BLOC, (c + 1) * BLOC)
        in_maps.append(
            {
                "memory": np.ascontiguousarray(memory[sl]),
                "read_vectors": np.ascontiguousarray(read_vectors[sl]),
                "read_strengths": np.ascontiguousarray(read_strengths[sl]),
                "identity": identity,
                "ones": ones_m,
            }
        )

    global last_results
    last_results = run_bass_kernel_spmd(nc, in_maps, list(range(NCORES)))
    res = last_results.results
    outs = []
    for c in range(NCORES):
        # (BLOC, 128, R, T) f16 * 2^15; tau = g*TPG + t; n = g*4096 + p*32 + t
        o = np.asarray(res[c]["out"]).astype(np.float32) / OUT_SCALE
        o = o.reshape(BLOC, 128, R, NG, TPG).transpose(0, 3, 1, 4, 2)
        outs.append(o.reshape(BLOC, N, R))
    return np.concatenate(outs, axis=0)


# revision 60
# speedup vs baseline: 1.3672x; 1.3672x over previous
"""Content-based addressing read (DNC-style) for Trainium2.

Computes softmax_n( strengths[r] * cos_sim(memory[b,n,:], read_vectors[b,:,r]) )
for B=16, N=32768, W=128, R=8, sharded batch-parallel across 8 NeuronCores
(2 batches per core).

v18: norm reduction moved onto the PE.
  - The w-reduction for ||mem_n||^2 no longer runs as squares+fold-tree on
    the vector engines. Instead the *drained* memT chunks are squared
    (GpSimd/ACT/DVE rotation, SBUF->SBUF) and a per-tile ones-column matmul
    accumulates sum_w memT^2 directly into PSUM in [p, t] layout (f32
    accumulate — better precision than the old f16 fold tree). This removes
    the entire n-major square pass + fd1/fd2/fd3/reduce chain from DVE/ACT.
  - norm matmuls are deferred one group so the PE queue never waits on the
    square producers; they also keep the PE busier (p-state ramp).
  - inv_nrm = ACT Abs_reciprocal_sqrt straight from the norm PSUM tile.
  - scores stored [128, R, T]: the sim-PSUM drain is a fused DVE multiply
    by inv_nrm; s1 reduce is contiguous; output written f16 scaled by 2^15
    (host rescales), quarters issued on sync/scalar/gpsimd HWDGE queues.
  - gpsimd casting DMA (f32 HBM -> f16 SBUF) in 2 half-group pieces with a
    5-group issue lookahead.
Softmax math stays fp32; no max subtraction (|scores| <= ~1.2); the
reference's +1e-8 is a provable fp32 no-op (normalizer ~128).

Output is stored in DRAM as (b, p, r, tau) f16*2^15 with n = g*4096 + p*32 + t,
tau = g*32 + t; the host rescales and re-transposes to (b, n, r) f32.
"""

import sys

for _p in ("/opt/trn_rl_repo",):
    if _p not in sys.path:
        sys.path.insert(0, _p)

from contextlib import ExitStack

import numpy as np

import concourse.bass as bass
import concourse.bacc as bacc
import concourse.tile as tile
from concourse import mybir
from concourse import bass_isa
from concourse.bass_utils import run_bass_kernel_spmd

F32 = mybir.dt.float32
F16 = mybir.dt.float16
AF = mybir.ActivationFunctionType

B, N, W, R = 16, 32768, 128, 8
NCORES = 8
BLOC = B // NCORES          # batches per core
T = N // 128                # 256 n-tiles of 128 per batch
NG = 8                      # DMA groups per batch
TPG = T // NG               # 32 tiles per group (4096 n, 2MB)
CH = 8                      # tiles per PSUM transpose chunk (1024 cols)
NCH = TPG // CH             # chunks per group
NSTEP = BLOC * NG           # 16 flat steps

# ---- tuning knobs ----
MEMT_DRAIN = "vs"           # rotation for memT PSUM->SBUF drains (no gp: PSUM)
SQ_ROT = "gad"              # rotation for memT^2 squares (g=GpSimd a=ACT d=DVE)
DMA_SPLIT = 2               # casting DMAs per group (earlier consumer wakeup)
DMA_AHEAD = 5               # DMA issue lookahead (must be < IN_BUFS - 2)
IN_BUFS = 10
OUT_SCALE = 32768.0         # output written f16 * 2^15; host divides


def build_program():
    nc = bacc.Bacc("TRN2", target_bir_lowering=False, debug=False, num_devices=NCORES)

    mem = nc.dram_tensor("memory", [BLOC, N, W], F32, kind="ExternalInput").ap()
    rv = nc.dram_tensor("read_vectors", [BLOC, W, R], F32, kind="ExternalInput").ap()
    rs = nc.dram_tensor("read_strengths", [BLOC, R], F32, kind="ExternalInput").ap()
    ident = nc.dram_tensor("identity", [128, 128], F32, kind="ExternalInput").ap()
    ones = nc.dram_tensor("ones", [128, 128], F32, kind="ExternalInput").ap()
    out = nc.dram_tensor("out", [BLOC, 128, R, T], F16, kind="ExternalOutput").ap()

    with ExitStack() as ctx:
        tc = ctx.enter_context(tile.TileContext(nc))

        const_pool = ctx.enter_context(tc.tile_pool(name="const", bufs=1))
        id_t = const_pool.tile([128, 128], F32)
        nc.sync.dma_start(id_t[:], ident)
        ones_t = const_pool.tile([128, 128], F32)
        nc.sync.dma_start(ones_t[:], ones)
        id_h = const_pool.tile([128, 128], F16)
        nc.scalar.copy(id_h[:], id_t[:])
        ones_h = const_pool.tile([128, 128], F16)
        nc.scalar.copy(ones_h[:], ones_t[:])

        in_pool = ctx.enter_context(tc.tile_pool(name="mem_in", bufs=IN_BUFS))
        mtps_pool = ctx.enter_context(tc.tile_pool(name="mtps", bufs=3, space="PSUM"))
        mt_pool = ctx.enter_context(tc.tile_pool(name="mt", bufs=6))
        mtq_pool = ctx.enter_context(tc.tile_pool(name="mtq", bufs=9))
        scps_pool = ctx.enter_context(tc.tile_pool(name="scps", bufs=3, space="PSUM"))
        nrm_pool = ctx.enter_context(tc.tile_pool(name="nrm", bufs=1, space="PSUM"))
        rtps_pool = ctx.enter_context(tc.tile_pool(name="rtps", bufs=1, space="PSUM"))
        smalls = ctx.enter_context(tc.tile_pool(name="smalls", bufs=2))
        score_pool = ctx.enter_context(tc.tile_pool(name="scores", bufs=2))
        scout_pool = ctx.enter_context(tc.tile_pool(name="scout", bufs=2))
        inv_pool = ctx.enter_context(tc.tile_pool(name="inv", bufs=2))

        state = {"drain_i": 0, "sq_i": 0}

        # per-batch / per-step state
        scores_t = [None] * BLOC
        inv_t = [None] * BLOC
        rvp_t = [None] * BLOC
        inv_tot_t = [None] * BLOC
        mem_tiles = {}   # flat step -> mem_g tile
        mtq_tiles = {}   # flat step -> list of squared memT chunks
        scps_tiles = {}  # flat step -> sim psum tile
        nrm_tiles = {}   # flat step -> norm psum tile

        def issue_dma(step):
            b, g = divmod(step, NG)
            mem_g = in_pool.tile([128, TPG, W], F16)
            src = mem[b, g * TPG * 128 : (g + 1) * TPG * 128, :].rearrange(
                "(p t) w -> p t w", p=128
            )
            hp = TPG // DMA_SPLIT
            for k in range(DMA_SPLIT):  # split: consumers wake at sub-group grain
                ts = slice(k * hp, (k + 1) * hp)
                nc.gpsimd.dma_start(mem_g[:, ts, :], src[:, ts, :])
            mem_tiles[step] = mem_g

        def rv_prep(b):
            rv_t = smalls.tile([128, R], F32)
            nc.sync.dma_start(rv_t[:], rv[b])
            rs_t = smalls.tile([1, R], F32)
            nc.sync.dma_start(rs_t[:], rs[b : b + 1, :])
            rs_h = smalls.tile([1, R], F16)
            nc.scalar.copy(rs_h[:], rs_t[:])

            rv2 = smalls.tile([128, R], F16)
            nc.vector.tensor_mul(rv2[:], rv_t[:], rv_t[:])
            nv2_ps = rtps_pool.tile([128, R], F32, tag="prep")
            nc.tensor.matmul(nv2_ps[:], ones_h[:], rv2[:], start=True, stop=True)
            inv_nv = smalls.tile([128, R], F32)
            nc.scalar.activation(inv_nv[:], nv2_ps[:], AF.Abs_reciprocal_sqrt)
            rsb_ps = rtps_pool.tile([128, R], F32, tag="prep")
            nc.tensor.matmul(
                rsb_ps[:], ones_h[0:1, :], rs_h[:], start=True, stop=True
            )
            factor = smalls.tile([128, R], F32)
            nc.vector.tensor_mul(factor[:], rsb_ps[:], inv_nv[:])
            rvp = smalls.tile([128, R], F32, tag="rvp")
            nc.vector.tensor_mul(rvp[:], rv_t[:], factor[:])
            rvp_h = smalls.tile([128, R], F16, tag="rvph")
            nc.scalar.copy(rvp_h[:], rvp[:])
            rvp_t[b] = rvp_h

        def emit_chunk(step, q, scps, sq_list):
            """transposes + memT drain + sim matmuls + memT^2 square."""
            b, g = divmod(step, NG)
            mem_g = mem_tiles[step]
            mt_ps = mtps_pool.tile([128, CH * 128], F16)
            for j in range(CH):
                tt = q * CH + j
                nc.tensor.transpose(
                    mt_ps[:, j * 128 : (j + 1) * 128],
                    mem_g[:, tt, :],
                    id_h[:],
                )
            mt_sb = mt_pool.tile([128, CH * 128], F16)
            de = MEMT_DRAIN[state["drain_i"] % len(MEMT_DRAIN)]
            state["drain_i"] += 1
            if de == "s":
                nc.scalar.copy(mt_sb[:], mt_ps[:])
            else:
                nc.vector.tensor_copy(mt_sb[:], mt_ps[:])

            for j in range(CH):
                tt = q * CH + j
                nc.tensor.matmul(
                    scps[:, tt * R : (tt + 1) * R],
                    mt_sb[:, j * 128 : (j + 1) * 128],
                    rvp_t[b][:],
                    start=True,
                    stop=True,
                )

            # square the drained chunk (SBUF->SBUF, any engine incl. GpSimd)
            mt_sq = mtq_pool.tile([128, CH * 128], F16)
            se = SQ_ROT[state["sq_i"] % len(SQ_ROT)]
            state["sq_i"] += 1
            if se == "g":
                nc.gpsimd.tensor_mul(mt_sq[:], mt_sb[:], mt_sb[:])
            elif se == "a":
                nc.scalar.square(mt_sq[:], mt_sb[:])
            else:
                nc.vector.tensor_mul(mt_sq[:], mt_sb[:], mt_sb[:])
            sq_list.append(mt_sq)

        def emit_norm_mms(step):
            """PE: per-tile ones-column matmuls accumulate ||mem_n||^2 into
            a [128(p), TPG(t)] PSUM tile. Deferred one group so the PE never
            waits on the square producers."""
            sq_list = mtq_tiles.pop(step)
            nrm_ps = nrm_pool.tile([128, TPG], F32)
            for q in range(NCH):
                mt_sq = sq_list[q]
                for j in range(CH):
                    tt = q * CH + j
                    nc.tensor.matmul(
                        nrm_ps[:, tt : tt + 1],
                        mt_sq[:, j * 128 : (j + 1) * 128],
                        ones_h[:, 0:1],
                        start=True,
                        stop=True,
                    )
            nrm_tiles[step] = nrm_ps

        def emit_inv(step):
            """inv_nrm = 1/sqrt(ss) straight from the norm PSUM tile."""
            b, g = divmod(step, NG)
            nrm_ps = nrm_tiles.pop(step)
            nc.scalar.activation(
                inv_t[b][:, g * TPG : (g + 1) * TPG],
                nrm_ps[:],
                AF.Abs_reciprocal_sqrt,
            )

        def emit_score_drain(step):
            """scores[:, :, tau] = scps * inv_nrm  (fused PSUM drain, DVE)."""
            b, g = divmod(step, NG)
            scps = scps_tiles.pop(step)
            scores = scores_t[b]
            ts = slice(g * TPG, (g + 1) * TPG)
            scps_v = scps[:].rearrange("p (t r) -> p t r", r=R).transpose([0, 2, 1])
            inv_b = (
                inv_t[b][:, ts].unsqueeze(1).broadcast_to([128, R, TPG])
            )
            nc.vector.tensor_mul(scores[:, :, ts], scps_v, inv_b)

        def softmax_tail_a(b):
            """exp + sum + 1/total (ACT + DVE + PE)."""
            scores = scores_t[b]          # [128, R, T] f32, pre-exp
            nc.scalar.activation(scores[:], scores[:], AF.Exp)
            s1 = smalls.tile([128, R], F32, tag="s1")
            nc.vector.reduce_sum(s1[:], scores[:], axis=mybir.AxisListType.X)
            s1_h = smalls.tile([128, R], F16)
            # pre-scale by 2^-15 so f16 output (x OUT_SCALE) stays normal-range
            nc.scalar.activation(s1_h[:], s1[:], AF.Copy, scale=1.0 / OUT_SCALE)
            tot_ps = rtps_pool.tile([128, R], F32, tag="prep")
            nc.tensor.matmul(tot_ps[:], ones_h[:], s1_h[:], start=True, stop=True)
            inv_tot = smalls.tile([128, R], F32, tag="invtot")
            nc.vector.reciprocal_approx_fast(inv_tot[:], tot_ps[:])
            inv_tot_t[b] = inv_tot

        def softmax_tail_b(b):
            """normalize quarters + output DMA on 4 queues (overlap the tail)."""
            scores = scores_t[b]
            inv_tot = inv_tot_t[b]
            scout = scout_pool.tile([128, R, T], F16, tag="scout")
            H = R // 4
            out_eng = [nc.sync, nc.scalar, nc.gpsimd, nc.sync]
            for h in range(4):
                hs = slice(h * H, (h + 1) * H)
                inv_b = inv_tot[:, hs].unsqueeze(2).broadcast_to([128, H, T])
                nc.vector.tensor_mul(scout[:, hs, :], scores[:, hs, :], inv_b)
                out_eng[h].dma_start(out[b, :, hs, :], scout[:, hs, :])

        # prologue: fill DMA pipeline, prep batch 0
        for s0 in range(DMA_AHEAD):
            issue_dma(s0)
        rv_prep(0)

        for step in range(NSTEP):
            b, g = divmod(step, NG)
            if g == 0:
                scores_new = score_pool.tile([128, R, T], F32, tag="scores")
                inv_new = inv_pool.tile([128, T], F32, tag="inv")
                scores_t[b] = scores_new
                inv_t[b] = inv_new
            if step + DMA_AHEAD < NSTEP:
                issue_dma(step + DMA_AHEAD)
            # batch b+1 rv-prep midway through batch b
            if g == NG - 3 and b + 1 < BLOC:
                rv_prep(b + 1)

            # previous group's norm matmuls: ready PE work at the head of the
            # queue (this group's transposes wait on DMA anyway)
            if step >= 1:
                emit_norm_mms(step - 1)
                emit_inv(step - 1)
            # fused score drain: ready DVE work (inv(step-2) emitted at step-1)
            if step >= 2:
                emit_score_drain(step - 2)
            if g == 2 and b > 0:
                softmax_tail_a(b - 1)

            scps = scps_pool.tile([128, TPG * R], F32)
            sq_list = []
            for q in range(NCH):
                emit_chunk(step, q, scps, sq_list)
            scps_tiles[step] = scps
            mtq_tiles[step] = sq_list
            mem_tiles.pop(step)

            if g == 3 and b > 0:
                softmax_tail_b(b - 1)

        # epilogue
        emit_norm_mms(NSTEP - 1)
        emit_inv(NSTEP - 1)
        emit_score_drain(NSTEP - 2)
        emit_score_drain(NSTEP - 1)
        softmax_tail_a(BLOC - 1)
        softmax_tail_b(BLOC - 1)

    nc.compile()
    return nc


_program = None
last_results = None


def _get_program():
    global _program
    if _program is None:
        _program = build_program()
    return _program


def kernel(memory, read_strengths, read_vectors):
    memory = np.asarray(memory, dtype=np.float32)
    read_strengths = np.asarray(read_strengths, dtype=np.float32)
    read_vectors = np.asarray(read_vectors, dtype=np.float32)

    nc = _get_program()
    identity = np.eye(128, dtype=np.float32)
    ones_m = np.ones((128, 128), dtype=np.float32)
    in_maps = []
    for c in range(NCORES):
        sl = slice(c * BLOC, (c + 1) * BLOC)
        in_maps.append(
            {
                "memory": np.ascontiguousarray(memory[sl]),
                "read_vectors": np.ascontiguousarray(read_vectors[sl]),
                "read_strengths": np.ascontiguousarray(read_strengths[sl]),
                "identity": identity,
                "ones": ones_m,
            }
        )

    global last_results
    last_results = run_bass_kernel_spmd(nc, in_maps, list(range(NCORES)))
    res = last_results.results
    outs = []
    for c in range(NCORES):
        # (BLOC, 128, R, T) f16 * 2^15; tau = g*TPG + t; n = g*4096 + p*32 + t
        o = np.asarray(res[c]["out"]).astype(np.float32) / OUT_SCALE
        o = o.reshape(BLOC, 128, R, NG, TPG).transpose(0, 3, 1, 4, 2)
        outs.append(o.reshape(BLOC, N, R))
    return np.concatenate(outs, axis=0)
